# revision 24
# baseline (speedup 1.0000x reference)
"""Trainium2 Bass kernel for nn_Attention_28243704938593.

Fused LayerNorm + QKV + block-causal (frame) attention + output projection.

Sharding (8 cores): data-parallel over batch (B=2) x tensor-parallel over
heads (12 heads -> 4 groups of 3). core_id = b*4 + g. Each core gets the
full sequence for its batch, computes its 3 heads end to end, and produces
a partial [T, DIM] output (its head-group's slice of the output projection).
The host sums the 4 partials per batch and adds b_out.

Device dataflow per core (matmul inputs bf16, fp32 PSUM accumulation):
  1. LN (fp32 bn_stats) -> xn bf16 -> PE-transpose -> xnT, pipelined per
     512-token run with the Q^T/K^T projection (heads 0,1) and V (head 0).
     PSUM evacuations ride the otherwise-idle ACT engine here.
  2. Attention per (head, qframe) over the EXACT allowed key span
     (block-causal at 196-token frames needs no masking): S^T = K^T.T @ Q^T
     per 128-key tile (K^T zero-padded to a 128-row contraction so every
     matmul runs in plain 128x128 mode - PE tiling-mode switches drain the
     array and measurably hurt on HW), one batched exp per 2-bank PSUM
     slot on ACT, attn@V with the query dim as M and a ones column
     appended to V producing the softmax denominator in the same PSUM
     accumulators, then a per-partition normalize (no cross-partition
     broadcasts anywhere). Software-pipelined: S-slots of step i
     interleave with attn@V chunks of step i-1, and deferred phase-1 work
     (head-2 QK block, V for heads 1/2) fills the PE while ACT grinds
     through the exps.
  3. Tail: head-2 O^T transposes pipelined with the output projection
     (heads stacked to K=128 contractions; zero-padded rows for head 2).

ln_gamma is folded into w_qkv on the host (exact). ln_beta and b_out are
zeros per the problem spec; b_out is still added on the host (ln_beta is
assumed zero - it cannot be folded post-hoc).
"""
import sys

if '/opt/trn_rl_repo' not in sys.path:
    sys.path.insert(0, '/opt/trn_rl_repo')

import contextlib
import numpy as np
import ml_dtypes

import concourse.bass as bass
import concourse.tile as tile
from concourse import bacc, mybir
from concourse.bass_utils import run_bass_kernel_spmd
from concourse.masks import make_identity

F32 = mybir.dt.float32
BF16 = mybir.dt.bfloat16
ALU = mybir.AluOpType
ACTF = mybir.ActivationFunctionType

# Problem constants (hardcoded; kernel.py must be self-contained).
B, T, DIM = 2, 3136, 768
HEADS, DH = 12, 64
FRAME, NF = 196, 16          # 196-token frames, 16 frames, block-causal
SCALE = DH ** -0.5
EPS = 1e-5

HPC = 3                      # heads per core
G = 4                        # head groups
P = 128
NT = (T + P - 1) // P        # 25 token tiles (24x128 + 64)
KCH = DIM // P               # 6 contraction chunks
TPAD = 3328                  # key padding: 128-wide S^T tiles + even slot counts
QKM = 3                      # qk weight M-tiles: [Q0;Q1] [K0;K1] [Q2;K2]
VCOL0 = QKM * P              # v columns start at 512 in the staged weight
WCOLS = VCOL0 + HPC * DH     # 704 staged qkv columns
NRUN = (T + 511) // 512      # 7 token runs of <=512
SLOT = 4                     # key-tiles per S-psum slot (SLOT//2 banks)

# kpad[:, h, :] holds head h's K^T zero-padded to 128 contraction rows:
# h0 -> [K0; 0], h1 -> [0; K1], h2 -> [0; K2]. The matching rhs has Q_h on
# the non-zero rows (block 0 for h0/h1; q2pad for h2), so S^T matmuls run
# in plain 128x128 mode (no row tiling -> no PE mode switches).


def build_nc(dbg=False, loop_n=1):
    nc = bacc.Bacc("TRN2", target_bir_lowering=False, debug=False)
    x_d = nc.dram_tensor("x", [T, DIM], F32, kind="ExternalInput")
    wqkv_d = nc.dram_tensor("wqkv", [DIM, WCOLS], BF16, kind="ExternalInput")
    wout_d = nc.dram_tensor("wout", [2 * P, DIM], BF16, kind="ExternalInput")
    out_d = nc.dram_tensor("out", [T, DIM], F32, kind="ExternalOutput")
    if dbg:
        dbg_xnT = nc.dram_tensor("dbg_xnT", [P, KCH, T], BF16, kind="ExternalOutput")
        dbg_qkT = nc.dram_tensor("dbg_qkT", [P, QKM, TPAD], BF16, kind="ExternalOutput")
        dbg_v = nc.dram_tensor("dbg_v", [P, NT, 196], BF16, kind="ExternalOutput")
        dbg_e = nc.dram_tensor("dbg_e", [P, 392], BF16, kind="ExternalOutput")
        dbg_oa = nc.dram_tensor("dbg_oa", [P, 130], F32, kind="ExternalOutput")
        dbg_osb = nc.dram_tensor("dbg_osb", [P, HPC, 2 * NF, DH], BF16,
                                 kind="ExternalOutput")

    with tile.TileContext(nc) as tc:
        with (tc.For_i(0, loop_n, 1) if loop_n > 1
              else contextlib.nullcontext()), \
             tc.tile_pool(name="big", bufs=1) as big:
            ident = big.tile([P, P], BF16)
            make_identity(nc, ident[:, :])
            eps_t = big.tile([P, 1], F32)
            nc.gpsimd.memset(eps_t[:, :], EPS)
            w_sb = big.tile([P, KCH, WCOLS], BF16)
            nc.gpsimd.dma_start(
                out=w_sb, in_=wqkv_d[:, :].rearrange("(c p) n -> p c n", p=P))
            wout_sb = big.tile([P, 2, DIM], BF16)
            nc.gpsimd.dma_start(
                out=wout_sb, in_=wout_d[:, :].rearrange("(c p) n -> p c n", p=P))

            xnT = big.tile([P, KCH, T], BF16)      # xn^T, chunk-major
            qkT = big.tile([P, QKM, TPAD], BF16)   # Q^T/K^T blocks, key-padded
            nc.gpsimd.memset(qkT[:, :, T:TPAD], 0.0)
            v_sb = big.tile([P, NT, 196], BF16)    # V token-major + ones cols
            for h in range(HPC):
                nc.gpsimd.memset(v_sb[:, :, h * 65 + 64:h * 65 + 65], 1.0)
            kpad = big.tile([P, HPC, TPAD], BF16)  # zero-padded K^T per head
            nc.gpsimd.memset(kpad[:, :, :], 0.0)
            q2pad = big.tile([P, TPAD], BF16)      # Q2 on rows 64:128
            nc.gpsimd.memset(q2pad[:, :], 0.0)
            osb = big.tile([P, HPC, 2 * NF, DH], BF16)   # normalized O, q-major
            oT01 = big.tile([P, T], BF16)          # O^T heads 0,1 stacked
            oT2 = big.tile([P, T], BF16)           # O^T head 2 (rows 0:64)
            nc.gpsimd.memset(oT2[64:P, :], 0.0)

            def run_tiles(r):
                return range(4 * r, min(4 * r + 4, NT))

            def emit_qk_block(m, r, pool, evac):
                r0, r1 = 512 * r, min(512 * r + 512, T)
                pq = pool.tile([P, 512], F32, tag="f" + pool.name, name="pq")
                for c in range(KCH):
                    nc.tensor.matmul(pq[:, 0:r1 - r0],
                                     w_sb[:, c, m * P:(m + 1) * P],
                                     xnT[:, c, r0:r1],
                                     start=(c == 0), stop=(c == KCH - 1))
                evac(out=qkT[:, m, r0:r1], in_=pq[:, 0:r1 - r0])

            def emit_v_tile(h, t, pool, evac):
                t0 = t * P
                sz = min(P, T - t0)
                pv = pool.tile([P, 512], F32, tag="f" + pool.name, name="pv")
                for c in range(KCH):
                    nc.tensor.matmul(
                        pv[0:sz, 0:DH],
                        xnT[:, c, t0:t0 + sz],
                        w_sb[:, c, VCOL0 + h * DH:VCOL0 + (h + 1) * DH],
                        start=(c == 0), stop=(c == KCH - 1))
                evac(out=v_sb[0:sz, t, h * 65:h * 65 + DH], in_=pv[0:sz, 0:DH])

            def emit_tr_pair(h, sp, pool, evac):
                """Transpose O slices (h, 2sp) and (h, 2sp+1) into oT."""
                tp = pool.tile([P, 256], BF16, tag="f" + pool.name, name="tp")
                for j in (0, 1):
                    nc.tensor.transpose(tp[0:DH, j * 98:(j + 1) * 98],
                                        osb[0:98, h, 2 * sp + j, :],
                                        ident[0:98, 0:98])
                dst, r0 = (oT01, h * DH) if h < 2 else (oT2, 0)
                evac(out=dst[r0:r0 + DH, sp * 196:(sp + 1) * 196],
                     in_=tp[0:DH, 0:196])

            # ---------- Phase 1: LN + transpose + QK(h0,h1) + V(h0) --------
            with tc.tile_pool(name="xp", bufs=4) as xp, \
                 tc.tile_pool(name="xnb", bufs=3) as xnbp, \
                 tc.tile_pool(name="stat", bufs=8) as statp, \
                 tc.tile_pool(name="ptp", bufs=3, space="PSUM") as ptp, \
                 tc.tile_pool(name="pqk", bufs=2, space="PSUM") as pqkp, \
                 tc.tile_pool(name="pvp", bufs=2, space="PSUM") as pvp:
                for r in range(NRUN):
                    for t in run_tiles(r):
                        t0 = t * P
                        sz = min(P, T - t0)
                        xt = xp.tile([P, DIM], F32)
                        nc.sync.dma_start(out=xt[:sz, :], in_=x_d[t0:t0 + sz, :])
                        st = statp.tile([P, 2, 6], F32, tag="st")
                        nc.vector.bn_stats(out=st[:sz, 0, :], in_=xt[:sz, 0:384])
                        nc.vector.bn_stats(out=st[:sz, 1, :], in_=xt[:sz, 384:768])
                        mv = statp.tile([P, 2], F32, tag="mv")
                        nc.vector.bn_aggr(out=mv[:sz, :], in_=st[:sz, :, :])
                        sq = statp.tile([P, 1], F32, tag="sq")
                        nc.scalar.activation(out=sq[:sz, :], in_=mv[:sz, 1:2],
                                             func=ACTF.Sqrt, bias=eps_t[:sz, :])
                        ri = statp.tile([P, 1], F32, tag="ri")
                        nc.vector.reciprocal(out=ri[:sz, :], in_=sq[:sz, :])
                        xnb = xnbp.tile([P, DIM], BF16)
                        eng = nc.gpsimd if t % 2 == 0 else nc.vector
                        eng.tensor_scalar(xnb[:sz, :], xt[:sz, :],
                                          mv[:sz, 0:1], ri[:sz, :],
                                          ALU.subtract, ALU.mult)
                        pt = ptp.tile([P, KCH, P], BF16)
                        for c in range(KCH):
                            nc.tensor.transpose(pt[:, c, 0:sz],
                                                xnb[:sz, c * P:(c + 1) * P],
                                                ident[0:sz, 0:sz])
                        nc.scalar.copy(out=xnT[:, :, t0:t0 + sz],
                                       in_=pt[:, :, 0:sz])
                    for m in (0, 1):
                        emit_qk_block(m, r, pqkp, nc.scalar.copy)
                    r0, r1 = 512 * r, min(512 * r + 512, T)
                    nc.gpsimd.tensor_copy(out=kpad[0:64, 0, r0:r1],
                                          in_=qkT[0:64, 1, r0:r1])
                    nc.gpsimd.tensor_copy(out=kpad[64:P, 1, r0:r1],
                                          in_=qkT[64:P, 1, r0:r1])
                    for t in run_tiles(r):
                        emit_v_tile(0, t, pvp, nc.scalar.copy)

            # ---------- Phase 2: attention with PE fillers -----------------
            with tc.tile_pool(name="sps", bufs=2, space="PSUM") as sps, \
                 tc.tile_pool(name="ops", bufs=1, space="PSUM") as ops, \
                 tc.tile_pool(name="fps", bufs=2, space="PSUM") as fps, \
                 tc.tile_pool(name="esb", bufs=3) as esb, \
                 tc.tile_pool(name="rsb", bufs=4) as rsb:

                def make_s_chunks(h, qf, e_t):
                    span = (qf + 1) * FRAME
                    n_kt = (span + P - 1) // P
                    n_kt += n_kt % 2          # even: exp merges per-bank pairs
                    q0 = qf * FRAME
                    rhs_q = (qkT[:, 0, q0:q0 + FRAME] if h < 2
                             else q2pad[:, q0:q0 + FRAME])

                    def mk(s0):
                        def emit():
                            cnt = min(SLOT, n_kt - s0)
                            slot = sps.tile([P, SLOT // 2, 512], F32, tag="s",
                                            name="slot")
                            for i in range(cnt):
                                kt = s0 + i
                                bank, half = i // 2, i % 2
                                nc.tensor.matmul(
                                    slot[:, bank, half * 196:half * 196 + 196],
                                    kpad[:, h, kt * P:(kt + 1) * P],
                                    rhs_q,
                                    start=True, stop=True)
                            fb = cnt // 2
                            if fb:
                                nc.scalar.activation(
                                    out=e_t[:, s0 * 196:(s0 + 2 * fb) * 196]
                                        .rearrange("p (b n) -> p b n", n=392),
                                    in_=slot[:, 0:fb, 0:392],
                                    func=ACTF.Exp, scale=SCALE)
                            if cnt % 2:
                                nc.scalar.activation(
                                    out=e_t[:, (s0 + cnt - 1) * 196:
                                            (s0 + cnt) * 196],
                                    in_=slot[:, fb, 0:196],
                                    func=ACTF.Exp, scale=SCALE)
                        return emit
                    return [mk(s0) for s0 in range(0, n_kt, SLOT)]

                def make_av_chunks(h, qf, e_t):
                    span = (qf + 1) * FRAME
                    n_kt = (span + P - 1) // P
                    state = {}

                    def mk(s0):
                        def emit():
                            if s0 == 0:
                                state["oa"] = ops.tile([P, 65], F32, tag="oa",
                                                       name="oa")
                                state["ob"] = ops.tile([P, 65], F32, tag="ob",
                                                       name="ob")
                            oa, ob = state["oa"], state["ob"]
                            for kt in range(s0, min(s0 + SLOT, n_kt)):
                                ksz = min(P, span - kt * P)
                                rv = v_sb[0:ksz, kt, h * 65:h * 65 + 65]
                                st_, sp_ = (kt == 0), (kt == n_kt - 1)
                                nc.tensor.matmul(
                                    oa[0:98, :],
                                    e_t[0:ksz, kt * 196:kt * 196 + 98], rv,
                                    start=st_, stop=sp_)
                                nc.tensor.matmul(
                                    ob[0:98, :],
                                    e_t[0:ksz, kt * 196 + 98:kt * 196 + 196],
                                    rv, start=st_, stop=sp_)
                            if s0 + SLOT >= n_kt:
                                if dbg and h == 0 and qf == 0:
                                    doa = rsb.tile([P, 130], F32, tag="doa",
                                                   name="doa")
                                    nc.vector.tensor_copy(out=doa[0:98, 0:65],
                                                          in_=oa[0:98, :])
                                    nc.vector.tensor_copy(out=doa[0:98, 65:130],
                                                          in_=ob[0:98, :])
                                    nc.gpsimd.dma_start(out=dbg_oa[0:98, :],
                                                        in_=doa[0:98, :])
                                rr = rsb.tile([P, 2], F32, tag="rr", name="rr")
                                nc.vector.reciprocal(out=rr[0:98, 0:1],
                                                     in_=oa[0:98, 64:65])
                                nc.vector.reciprocal(out=rr[0:98, 1:2],
                                                     in_=ob[0:98, 64:65])
                                nc.vector.tensor_scalar_mul(
                                    osb[0:98, h, 2 * qf, :],
                                    oa[0:98, 0:DH], rr[0:98, 0:1])
                                nc.vector.tensor_scalar_mul(
                                    osb[0:98, h, 2 * qf + 1, :],
                                    ob[0:98, 0:DH], rr[0:98, 1:2])
                        return emit
                    return [mk(s0) for s0 in range(0, n_kt, SLOT)]

                # deferred-work fillers per head window (PE work + DVE evac)
                vcopy = nc.vector.tensor_copy

                def emit_qk2(r):
                    emit_qk_block(2, r, fps, vcopy)
                    r0, r1 = 512 * r, min(512 * r + 512, T)
                    nc.gpsimd.tensor_copy(out=kpad[64:P, 2, r0:r1],
                                          in_=qkT[64:P, 2, r0:r1])
                    nc.sync.dma_start(out=q2pad[64:P, r0:r1],
                                      in_=qkT[0:64, 2, r0:r1])

                f_win = {
                    0: [lambda r=r: emit_qk2(r) for r in range(NRUN)]
                       + [lambda t=t: emit_v_tile(1, t, fps, vcopy)
                          for t in range(NT)],
                    1: [lambda t=t: emit_v_tile(2, t, fps, vcopy)
                        for t in range(NT)],
                    2: [],
                }
                f_done = {0: 0, 1: 0, 2: 0}

                def pop_fillers(h, step_in_win):
                    fl = f_win[h]
                    want = (len(fl) * step_in_win) // (NF - 1)
                    while f_done[h] < min(want, len(fl)):
                        fl[f_done[h]]()
                        f_done[h] += 1

                seq = [(h, qf) for h in range(HPC) for qf in range(NF)]
                prev_av = []
                for (h, qf) in seq:
                    e_t = esb.tile([P, 26 * FRAME], BF16, tag="e", name="e_t")
                    s_chunks = make_s_chunks(h, qf, e_t)
                    for j in range(max(len(s_chunks), len(prev_av))):
                        if j < len(s_chunks):
                            s_chunks[j]()
                        if j < len(prev_av):
                            prev_av[j]()
                    if qf >= 1:
                        # first step of each window: window deps not ready yet
                        pop_fillers(h, qf)
                    if dbg and h == 0 and qf == 0:
                        nc.gpsimd.dma_start(out=dbg_e[:, :], in_=e_t[:, 0:392])
                    prev_av = make_av_chunks(h, qf, e_t)
                for c in prev_av:
                    c()

            if dbg:
                nc.gpsimd.dma_start(out=dbg_xnT[:, :, :], in_=xnT[:, :, :])
                nc.gpsimd.dma_start(out=dbg_qkT[:, :, :], in_=qkT[:, :, :])
                nc.gpsimd.dma_start(out=dbg_v[:, :, :], in_=v_sb[:, :, :])
                nc.gpsimd.dma_start(out=dbg_osb[:, :, :, :], in_=osb[:, :, :, :])

            # ---------- Tail: head-2 transposes + output projection --------
            with tc.tile_pool(name="tpp", bufs=2, space="PSUM") as tpp, \
                 tc.tile_pool(name="pon", bufs=3, space="PSUM") as pon, \
                 tc.tile_pool(name="osg", bufs=4) as osg:
                for h in (0, 1):
                    for sp in range(NF):
                        emit_tr_pair(h, sp, tpp, nc.vector.tensor_copy)
                sp_done = 0
                for t in range(NT):
                    t0 = t * P
                    sz = min(P, T - t0)
                    while sp_done * 196 < t0 + sz and sp_done < NF:
                        emit_tr_pair(2, sp_done, tpp, nc.vector.tensor_copy)
                        sp_done += 1
                    po = pon.tile([P, 2, 512], F32)
                    for nhalf in range(2):
                        n0 = nhalf * 384
                        nc.tensor.matmul(po[0:sz, nhalf, 0:384],
                                         oT01[:, t0:t0 + sz],
                                         wout_sb[:, 0, n0:n0 + 384],
                                         start=True, stop=False)
                        nc.tensor.matmul(po[0:sz, nhalf, 0:384],
                                         oT2[:, t0:t0 + sz],
                                         wout_sb[:, 1, n0:n0 + 384],
                                         start=False, stop=True)
                    og = osg.tile([P, DIM], F32)
                    nc.vector.tensor_copy(out=og[0:sz, 0:384],
                                          in_=po[0:sz, 0, 0:384])
                    nc.scalar.copy(out=og[0:sz, 384:768],
                                   in_=po[0:sz, 1, 0:384])
                    nc.sync.dma_start(out=out_d[t0:t0 + sz, :], in_=og[0:sz, :])

    nc.finalize()
    return nc


_NC_CACHE = None


def _get_nc():
    global _NC_CACHE
    if _NC_CACHE is None:
        _NC_CACHE = build_nc()
    return _NC_CACHE


def _make_in_maps(x, ln_gamma, w_qkv, w_out):
    """Build the 8 per-core input maps (host-side sharding)."""
    w_eff = (np.asarray(ln_gamma, np.float32)[:, None]
             * np.asarray(w_qkv, np.float32))       # [768, 2304]
    in_maps = []
    for b in range(B):
        for g in range(G):
            c0 = g * HPC * DH
            q = w_eff[:, c0:c0 + HPC * DH]              # [768, 192]
            k = w_eff[:, DIM + c0:DIM + c0 + HPC * DH]
            v = w_eff[:, 2 * DIM + c0:2 * DIM + c0 + HPC * DH]
            # M-tiles: [Q0;Q1] [K0;K1] [Q2;K2] [K2;Q2] then V
            wq = np.concatenate(
                [q[:, 0:128], k[:, 0:128],
                 q[:, 128:192], k[:, 128:192], v], axis=1)   # [768, 576]
            wo = np.zeros((2 * P, DIM), np.float32)
            wo[0:192] = np.asarray(w_out, np.float32)[c0:c0 + HPC * DH, :]
            in_maps.append({
                "x": np.ascontiguousarray(np.asarray(x, np.float32)[b]),
                "wqkv": np.ascontiguousarray(wq).astype(ml_dtypes.bfloat16),
                "wout": np.ascontiguousarray(wo).astype(ml_dtypes.bfloat16),
            })
    return in_maps


def run_sharded(x, ln_gamma, w_qkv, w_out, b_out, trace=False, **spmd_kwargs):
    in_maps = _make_in_maps(x, ln_gamma, w_qkv, w_out)
    res = run_bass_kernel_spmd(_get_nc(), in_maps, core_ids=list(range(2 * G)),
                               trace=trace, **spmd_kwargs)
    parts = [r["out"] for r in res.results]
    out = np.stack([sum(parts[b * G + 1:(b + 1) * G], parts[b * G])
                    for b in range(B)])
    out = out + np.asarray(b_out, np.float32)[None, None, :]
    return out.astype(np.float32), res


def kernel(x, ln_gamma, ln_beta, w_qkv, w_out, b_out, mask):
    # ln_beta is zeros and mask is the fixed block-causal frame mask per the
    # problem spec; both are hardcoded into the device program.
    out, _ = run_sharded(x, ln_gamma, w_qkv, w_out, b_out)
    return out
